# revision 1
# baseline (speedup 1.0000x reference)
"""Distributed Trainium2 kernel for fused multi-head attention
(QKV proj + RoPE + causal/key-padded SDPA + out-proj + bias).

Sharding: tensor-parallel over heads across 8 cores (2 heads/core, both
batches on every core).  After attention, per-(head,batch) AllToAlls
convert head-shards into sequence-row-shards so the output projection is
computed locally per row slice; the host concatenates the 8 row slices.
The AllToAlls are issued as soon as each (head, batch) finishes attention,
overlapping the next attention pair and the projection's first
accumulation half.

All matmuls run in float32r (full-rate fp32 on the PE at moving dim >= 256,
~2e-4 relative precision measured on HW).  Softmax skips max-subtraction
(scores are O(10) here, far from fp32 exp overflow); key-padding masks are
an additive -30000 activation bias (per-partition, from L), the causal mask
is a post-exp affine_select zero fill (compile-time pattern).
"""

import numpy as np

import concourse.bacc as bacc
import concourse.bass as bass
import concourse.mybir as mybir
import concourse.tile as tile
from concourse import bass_utils

B, N, D, NH = 2, 2048, 2048, 16
HD = 128               # head dim
NCORES = 8
HL = NH // NCORES      # heads per core = 2
DL = HL * HD           # local model cols = 256
NS = N // NCORES       # output row slice per core = 256
HALF = HD // 2
ET = D // HD           # 16 contraction tiles
NT = N // HD           # 16 seq tiles of 128
NCH = N // 512         # 4 free-dim chunks of 512
SCALE = 1.0 / float(np.sqrt(HD))
NEGBIG = -30000.0
ROPE_BASE = 10000.0

F32 = mybir.dt.float32
F32R = mybir.dt.float32r
I32 = mybir.dt.int32

_CACHE = {}


def build():
    if "nc" in _CACHE:
        return _CACHE["nc"]
    nc = bacc.Bacc("TRN2", target_bir_lowering=False, debug=False,
                   num_devices=NCORES)
    xT = nc.dram_tensor("xT", [B, D, N], F32R, kind="ExternalInput")
    wqkvT = nc.dram_tensor("wqkvT", [3, D, DL], F32R, kind="ExternalInput")
    wpT = nc.dram_tensor("wpT", [D, D], F32R, kind="ExternalInput")
    bp = nc.dram_tensor("bp", [1, D], F32, kind="ExternalInput")
    cosT = nc.dram_tensor("cosT", [HD, N], F32, kind="ExternalInput")
    sinT = nc.dram_tensor("sinT", [HD, N], F32, kind="ExternalInput")
    Lw = nc.dram_tensor("Lw", [1, B], I32, kind="ExternalInput")
    onesv = nc.dram_tensor("onesv", [HD, 1], F32R, kind="ExternalInput")
    out = nc.dram_tensor("out", [B, NS, D], F32, kind="ExternalOutput")

    AF = mybir.ActivationFunctionType
    ALU = mybir.AluOpType

    with tile.TileContext(nc) as tc:
        with tc.tile_pool(name="persist", bufs=1) as pp, \
             tc.tile_pool(name="dram", bufs=1, space="DRAM") as dp:
            # per-(b,h) scratch so attention can start while later batches
            # are still in the QKV phase
            qs = [[dp.tile([HD, N], F32R, name=f"qs{b}{h}")
                   for h in range(HL)] for b in range(B)]
            ks = [[dp.tile([HD, N], F32R, name=f"ks{b}{h}")
                   for h in range(HL)] for b in range(B)]
            # one AllToAll per (h, b): issued right after that pair's
            # attention completes
            ca = [[dp.tile([NCORES, HD, NS], F32R, name=f"ca{h}{b}")
                   for b in range(B)] for h in range(HL)]
            cb = [[dp.tile([NCORES, HD, NS], F32R, name=f"cb{h}{b}")
                   for b in range(B)] for h in range(HL)]

            ones = pp.tile([HD, 1], F32R)
            nc.sync.dma_start(ones[:], onesv[:])

            # key-padding additive bias per (partition=j%128, batch, jtile)
            iota = pp.tile([HD, NT], I32)
            nc.gpsimd.iota(iota[:], pattern=[[HD, NT]], base=0,
                           channel_multiplier=1)
            iotaf = pp.tile([HD, NT], F32)
            nc.vector.tensor_copy(iotaf[:], iota[:])
            lsb = pp.tile([1, B], I32)
            nc.sync.dma_start(lsb[:], Lw[:])
            lf = pp.tile([1, B], F32)
            nc.vector.tensor_copy(lf[:], lsb[:])
            lb = pp.tile([HD, B], F32)
            nc.gpsimd.partition_broadcast(lb[:], lf[:])
            kpad = pp.tile([HD, B, NT], F32)
            for b in range(B):
                nc.vector.tensor_scalar(kpad[:, b, :], iotaf[:], lb[:, b:b + 1],
                                        NEGBIG, ALU.is_ge, ALU.mult)

            # bias row broadcast to all 128 partitions
            bp1 = pp.tile([1, D], F32)
            nc.sync.dma_start(bp1[:], bp[:])
            bpb = pp.tile([HD, D], F32)
            nc.gpsimd.partition_broadcast(bpb[:], bp1[:])

            # rope tables duplicated across both partition halves
            cosb = pp.tile([HD, N], F32)
            nc.sync.dma_start(cosb[:], cosT[:])
            sinb = pp.tile([HD, N], F32)
            nc.sync.dma_start(sinb[:], sinT[:])

            # first proj weight chunk prefetches during earlier phases;
            # V stays resident in SBUF across phases 1-2 (no DRAM roundtrip)
            pwctx = tc.tile_pool(name="projw0", bufs=1)
            pw0 = pwctx.__enter__()
            wpt0 = pw0.tile([HD, ET, 512], F32R, name="wpt0")
            nc.sync.dma_start(
                wpt0[:], wpT[:, 0:512].rearrange("(t p) f -> p t f", p=HD))
            vsctx = tc.tile_pool(name="vres", bufs=1)
            vsp = vsctx.__enter__()
            vsb = [vsp.tile([HD, NT, DL], F32R, name=f"vsb{b}")
                   for b in range(B)]

            # ---------------- Phase 1: QKV projection + RoPE ----------------
            with tc.tile_pool(name="wqkv", bufs=1) as wqp, \
                 tc.tile_pool(name="ph1x", bufs=20) as xp, \
                 tc.tile_pool(name="ph1s", bufs=3) as sp, \
                 tc.tile_pool(name="ph1p", bufs=2, space="PSUM") as pq:
                wq = wqp.tile([HD, ET, DL], F32R, tag="wq")
                wk = wqp.tile([HD, ET, DL], F32R, tag="wk")
                wv = wqp.tile([HD, ET, DL], F32R, tag="wv")
                for w, i in ((wq, 0), (wk, 1), (wv, 2)):
                    nc.sync.dma_start(
                        w[:], wqkvT[i].rearrange("(t p) d -> p t d", p=HD))

                for b in range(B):
                    for c4 in range(NCH):
                        nsl = slice(c4 * 512, (c4 + 1) * 512)
                        xts = []
                        for et in range(ET):
                            xt = xp.tile([HD, 512], F32R, tag="xt",
                                         name=f"xt{et}")
                            nc.sync.dma_start(
                                xt[:], xT[b, et * HD:(et + 1) * HD, nsl])
                            xts.append(xt)
                        # two 4-bank sub-iterations (one per head) so the
                        # PSUM pool double-buffers and the PE never waits
                        # for the rope/copy epilogue
                        for h in range(HL):
                            psq = pq.tile([HD, 512], F32, tag="pq")
                            psk = pq.tile([HD, 512], F32, tag="pk")
                            psv = [pq.tile([HD, DL], F32, tag=f"pv{i}",
                                           name=f"psv{i}") for i in range(2)]
                            for et in range(ET):
                                st = (et == 0)
                                en = (et == ET - 1)
                                nc.tensor.matmul(
                                    psq[:], wq[:, et, h * HD:(h + 1) * HD],
                                    xts[et][:], start=st, stop=en)
                                nc.tensor.matmul(
                                    psk[:], wk[:, et, h * HD:(h + 1) * HD],
                                    xts[et][:], start=st, stop=en)
                                for i in range(2):
                                    s4 = 2 * h + i
                                    nc.tensor.matmul(
                                        psv[i][:],
                                        xts[et][:, s4 * HD:(s4 + 1) * HD],
                                        wv[:, et, :], start=st, stop=en)
                            # copy-first RoPE: free the PSUM bank after one
                            # copy, rotate in SBUF (tables half-duplicated so
                            # every tensor_tensor has equal partition bases)
                            for src, dst in ((psq, qs[b][h]),
                                             (psk, ks[b][h])):
                                stg = sp.tile([HD, 512], F32R, tag="stg")
                                t12 = sp.tile([HD, 512], F32, tag="t12")
                                nc.vector.tensor_copy(stg[:], src[:])
                                nc.vector.tensor_mul(
                                    t12[:HALF, :], stg[HALF:, :],
                                    sinb[HALF:, nsl])
                                nc.vector.tensor_mul(
                                    t12[HALF:, :], stg[:HALF, :],
                                    sinb[:HALF, nsl])
                                nc.vector.tensor_mul(stg[:], stg[:],
                                                     cosb[:, nsl])
                                nc.vector.tensor_sub(stg[:HALF, :],
                                                     stg[:HALF, :],
                                                     t12[:HALF, :])
                                nc.vector.tensor_add(stg[HALF:, :],
                                                     stg[HALF:, :],
                                                     t12[HALF:, :])
                                nc.sync.dma_start(dst[:, nsl], stg[:])
                            for i in range(2):
                                nc.vector.tensor_copy(
                                    vsb[b][:, c4 * 4 + 2 * h + i, :],
                                    psv[i][:])

            # ------------- Phase 2: attention per (h, b) + AllToAll --------
            with tc.tile_pool(name="att", bufs=2) as ap_, \
                 tc.tile_pool(name="atts", bufs=6) as sp2, \
                 tc.tile_pool(name="attp", bufs=4, space="PSUM") as pq2, \
                 tc.tile_pool(name="attpo", bufs=2, space="PSUM") as pq3:
                for h in range(HL):
                    for b in range(B):
                        qt = ap_.tile([HD, N], F32R, tag="qt")
                        nc.sync.dma_start(qt[:], qs[b][h][:])
                        kt = ap_.tile([HD, N], F32R, tag="kt")
                        nc.sync.dma_start(kt[:], ks[b][h][:])
                        vt = vsb[b][:, :, h * HD:(h + 1) * HD]
                        for c4 in range(NCH):
                            nsl = slice(c4 * 512, (c4 + 1) * 512)
                            pso = pq3.tile([HD, 512], F32, tag="pso")
                            psd = pq3.tile([1, 512], F32, tag="psd")
                            njt = 4 * c4 + 4
                            for jt in range(njt):
                                # diagonal tiles (jt = 4*c4+r, r>0) have no
                                # valid columns below 128*r: stream only the
                                # live range through PE/ACT/DVE
                                r = jt - 4 * c4
                                lo = 128 * r if r > 0 else 0
                                w = 512 - lo
                                pss = pq2.tile([HD, 512], F32, tag="pss")
                                nc.tensor.matmul(
                                    pss[:, lo:], kt[:, jt * HD:(jt + 1) * HD],
                                    qt[:, c4 * 512 + lo:(c4 + 1) * 512],
                                    start=True, stop=True)
                                pt = sp2.tile([HD, 512], F32R, tag="pt")
                                nc.scalar.activation(
                                    pt[:, lo:], pss[:, lo:], AF.Exp,
                                    bias=kpad[:, b, jt:jt + 1], scale=SCALE)
                                if r >= 0 and jt >= 4 * c4:
                                    nc.gpsimd.affine_select(
                                        out=pt[:, lo:], in_=pt[:, lo:],
                                        compare_op=ALU.is_ge, fill=0.0,
                                        base=0,
                                        pattern=[[1, w]],
                                        channel_multiplier=-1)
                                nc.tensor.matmul(pso[:, lo:], vt[:, jt, :],
                                                 pt[:, lo:],
                                                 start=(jt == 0),
                                                 stop=(jt == njt - 1))
                                nc.tensor.matmul(psd[:, lo:], ones[:],
                                                 pt[:, lo:],
                                                 start=(jt == 0),
                                                 stop=(jt == njt - 1))
                            rec = sp2.tile([1, 512], F32, tag="rec")
                            nc.vector.reciprocal(rec[:], psd[:])
                            rb = sp2.tile([HD, 512], F32, tag="rb")
                            nc.gpsimd.partition_broadcast(rb[:], rec[:])
                            ou = sp2.tile([HD, 512], F32R, tag="ou")
                            nc.vector.tensor_mul(ou[:], pso[:], rb[:])
                            for r in range(2):
                                nc.sync.dma_start(
                                    ca[h][b][2 * c4 + r],
                                    ou[:, r * NS:(r + 1) * NS])
                        # exchange this (head, batch) while the next pair
                        # computes
                        nc.gpsimd.collective_compute(
                            "AllToAll", mybir.AluOpType.bypass,
                            replica_groups=[list(range(NCORES))],
                            ins=[ca[h][b].opt()], outs=[cb[h][b].opt()])

            vsctx.__exit__(None, None, None)

            # ---------------- Phase 3: output projection ----------------
            with tc.tile_pool(name="proj", bufs=1) as pj, \
                 tc.tile_pool(name="projw", bufs=2) as pw, \
                 tc.tile_pool(name="projs", bufs=4) as po, \
                 tc.tile_pool(name="projp", bufs=4, space="PSUM") as pq4:
                # asb[h][p, b, s, n] = cb[h][b][s, p, n]
                asb = [pj.tile([HD, B, NCORES, NS], F32R, name=f"asb{h}")
                       for h in range(HL)]
                for h in range(HL):
                    for b in range(B):
                        for s in range(NCORES):
                            nc.sync.dma_start(asb[h][:, b, s], cb[h][b][s])
                for f4 in range(NCH):
                    fsl = slice(f4 * 512, (f4 + 1) * 512)
                    if f4 == 0:
                        wpt = wpt0
                    else:
                        wpt = pw.tile([HD, ET, 512], F32R, tag="wpt")
                        nc.sync.dma_start(
                            wpt[:],
                            wpT[:, fsl].rearrange("(t p) f -> p t f", p=HD))
                    for b in range(B):
                        for m in range(NS // HD):
                            psp = pq4.tile([HD, 512], F32, tag="psp")
                            # h=0 tiles first: they are exchanged earlier, so
                            # accumulation can begin while h=1 is in flight
                            for gi in range(ET):
                                h, s = gi // NCORES, gi % NCORES
                                nc.tensor.matmul(
                                    psp[:],
                                    asb[h][:, b, s, m * HD:(m + 1) * HD],
                                    wpt[:, 2 * s + h, :],
                                    start=(gi == 0), stop=(gi == ET - 1))
                            ot = po.tile([HD, 512], F32, tag="ot")
                            nc.vector.tensor_add(ot[:], psp[:], bpb[:, fsl])
                            nc.sync.dma_start(
                                out[b, m * HD:(m + 1) * HD, fsl], ot[:])
            pwctx.__exit__(None, None, None)

    nc.compile()
    _CACHE["nc"] = nc
    return nc


def _prep_inputs(x, Wqkv, Wproj, bproj, L):
    x = np.asarray(x, np.float32)
    Wqkv = np.asarray(Wqkv, np.float32)
    Wproj = np.asarray(Wproj, np.float32)
    bproj = np.asarray(bproj, np.float32)
    L = np.asarray(L, np.int32)

    xT = np.ascontiguousarray(x.transpose(0, 2, 1))
    wpT = np.ascontiguousarray(Wproj.T)
    inv = 1.0 / (ROPE_BASE ** (np.arange(0, HD, 2, dtype=np.float32) / HD))
    ang = np.arange(N, dtype=np.float32)[:, None] * inv[None, :]
    cos1 = np.cos(ang).T.astype(np.float32)          # [64, N]
    sin1 = np.sin(ang).T.astype(np.float32)
    cosT = np.ascontiguousarray(np.vstack([cos1, cos1]))   # [128, N]
    sinT = np.ascontiguousarray(np.vstack([sin1, sin1]))
    Lw = L.reshape(1, B).astype(np.int32)
    bp = bproj.reshape(1, D)

    in_maps = []
    for c in range(NCORES):
        sl = slice(c * DL, (c + 1) * DL)
        w3 = np.stack([
            np.ascontiguousarray(Wqkv[0 * D:1 * D][sl].T),
            np.ascontiguousarray(Wqkv[1 * D:2 * D][sl].T),
            np.ascontiguousarray(Wqkv[2 * D:3 * D][sl].T),
        ])
        in_maps.append({
            "xT": xT, "wqkvT": w3, "wpT": wpT, "bp": bp,
            "cosT": cosT, "sinT": sinT, "Lw": Lw,
            "onesv": np.ones((HD, 1), np.float32),
        })
    return in_maps


def run(x, Wqkv, Wproj, bproj, L, trace=False, tmpdir=None):
    nc = build()
    in_maps = _prep_inputs(x, Wqkv, Wproj, bproj, L)
    kw = {}
    if tmpdir is not None:
        kw["tmpdir"] = tmpdir
    res = bass_utils.run_bass_kernel_spmd(
        nc, in_maps, core_ids=list(range(NCORES)), trace=trace, **kw)
    full = np.empty((B, N, D), np.float32)
    for c in range(NCORES):
        full[:, c * NS:(c + 1) * NS, :] = res.results[c]["out"]
    return full, res


def kernel(x, Wqkv, Wproj, bproj, L, n_heads):
    assert int(n_heads) == NH
    full, _ = run(x, Wqkv, Wproj, bproj, L, trace=False)
    return full



# revision 8
# speedup vs baseline: 1.1482x; 1.1482x over previous
"""Distributed Trainium2 kernel for fused multi-head attention
(QKV proj + RoPE + causal/key-padded SDPA + out-proj + bias).

Sharding: tensor-parallel over heads across 8 cores (2 heads/core, both
batches on every core).  After attention, per-(head,batch) AllToAlls
convert head-shards into sequence-row-shards so the output projection is
computed locally per row slice; the host concatenates the 8 row slices.

v2 restructure vs baseline:
 - Q/K stay SBUF-resident (no DRAM roundtrip between QKV and attention).
 - QKV weights stream per-et slice so the first matmul starts ~2us in
   (baseline waited ~60us for three monolithic rearranged weight DMAs).
 - Softmax denominators accumulate on DVE (bf16 2x) + one ones-matmul per
   chunk instead of a ones-matmul per key tile (saves ~70us of PE).
 - Causal masking of the diagonal 128x128 sub-block via a static bf16
   triangular-mask multiply on DVE (replaces per-tile gpsimd affine_select).
 - reciprocal_approx_fast (~5x cheaper than DVE reciprocal, 18-bit).
 - V / attn*V / AllToAll / out-projection all in bf16: halves A2A wire and
   proj weight DMA, makes LDWEIGHTS fully hidden; rel err ~5e-3 << 2e-2.
"""

import numpy as np
import ml_dtypes

import concourse.bacc as bacc
import concourse.bass as bass
import concourse.mybir as mybir
import concourse.tile as tile
from concourse import bass_utils

B, N, D, NH = 2, 2048, 2048, 16
HD = 128               # head dim
NCORES = 8
HL = NH // NCORES      # heads per core = 2
DL = HL * HD           # local model cols = 256
NS = N // NCORES       # output row slice per core = 256
HALF = HD // 2
ET = D // HD           # 16 contraction tiles
NT = N // HD           # 16 seq tiles of 128
NCH = N // 512         # 4 free-dim chunks of 512
SCALE = 1.0 / float(np.sqrt(HD))
NEGBIG = -30000.0
ROPE_BASE = 10000.0

F32 = mybir.dt.float32
F32R = mybir.dt.float32r
BF16 = mybir.dt.bfloat16
I32 = mybir.dt.int32

_CACHE = {}


def build():
    if "nc" in _CACHE:
        return _CACHE["nc"]
    nc = bacc.Bacc("TRN2", target_bir_lowering=False, debug=False,
                   num_devices=NCORES)
    xT = nc.dram_tensor("xT", [B, D, N], F32R, kind="ExternalInput")
    wqT = nc.dram_tensor("wqT", [ET, HD, DL], F32R, kind="ExternalInput")
    wkT = nc.dram_tensor("wkT", [ET, HD, DL], F32R, kind="ExternalInput")
    wvT = nc.dram_tensor("wvT", [ET, HD, DL], F32R, kind="ExternalInput")
    wpTb = nc.dram_tensor("wpTb", [HD, ET, D], BF16, kind="ExternalInput")
    bp = nc.dram_tensor("bp", [1, D], F32, kind="ExternalInput")
    cosT = nc.dram_tensor("cosT", [HD, N], F32, kind="ExternalInput")
    sinT = nc.dram_tensor("sinT", [HD, N], F32, kind="ExternalInput")
    Lw = nc.dram_tensor("Lw", [1, B], I32, kind="ExternalInput")
    onesv = nc.dram_tensor("onesv", [HD, 1], BF16, kind="ExternalInput")
    trimT = nc.dram_tensor("trimT", [HD, HD], BF16, kind="ExternalInput")
    out = nc.dram_tensor("out", [B, NS, D], F32, kind="ExternalOutput")

    AF = mybir.ActivationFunctionType
    ALU = mybir.AluOpType

    with tile.TileContext(nc) as tc:
        with tc.tile_pool(name="persist", bufs=1) as pp, \
             tc.tile_pool(name="dram", bufs=1, space="DRAM") as dp:
            # one AllToAll per (h, b): issued right after that pair's
            # attention completes
            ca = [[dp.tile([NCORES, HD, NS], BF16, name=f"ca{h}{b}")
                   for b in range(B)] for h in range(HL)]
            cb = [[dp.tile([NCORES, HD, NS], BF16, name=f"cb{h}{b}")
                   for b in range(B)] for h in range(HL)]

            ones = pp.tile([HD, 1], BF16)
            nc.sync.dma_start(ones[:], onesv[:])
            trim = pp.tile([HD, HD], BF16)
            nc.sync.dma_start(trim[:], trimT[:])

            # key-padding additive bias per (partition=j%128, batch, jtile)
            iota = pp.tile([HD, NT], I32)
            nc.gpsimd.iota(iota[:], pattern=[[HD, NT]], base=0,
                           channel_multiplier=1)
            iotaf = pp.tile([HD, NT], F32)
            nc.vector.tensor_copy(iotaf[:], iota[:])
            lsb = pp.tile([1, B], I32)
            nc.sync.dma_start(lsb[:], Lw[:])
            lf = pp.tile([1, B], F32)
            nc.vector.tensor_copy(lf[:], lsb[:])
            lb = pp.tile([HD, B], F32)
            nc.gpsimd.partition_broadcast(lb[:], lf[:])
            kpad = pp.tile([HD, B, NT], F32)
            for b in range(B):
                nc.vector.tensor_scalar(kpad[:, b, :], iotaf[:], lb[:, b:b + 1],
                                        NEGBIG, ALU.is_ge, ALU.mult)

            # rope tables duplicated across both partition halves
            cosb = pp.tile([HD, N], F32)
            nc.sync.dma_start(cosb[:], cosT[:])
            sinb = pp.tile([HD, N], F32)
            nc.sync.dma_start(sinb[:], sinT[:])

            # SBUF-resident q/k (f32r) and v (bf16), per batch
            qkctx = tc.tile_pool(name="qk", bufs=2)
            qkp = qkctx.__enter__()
            vsctx = tc.tile_pool(name="vres", bufs=2)
            vsp = vsctx.__enter__()
            qsb = {}
            ksb = {}
            vsb = {}

            # ---------------- Phase 1: QKV projection + RoPE ----------------
            with tc.tile_pool(name="wqkv", bufs=1) as wqp, \
                 tc.tile_pool(name="ph1x", bufs=18) as xp, \
                 tc.tile_pool(name="ph1s", bufs=2) as sp, \
                 tc.tile_pool(name="ph1p", bufs=2, space="PSUM") as pq:
                wq = wqp.tile([HD, ET, DL], F32R, tag="wq")
                wk = wqp.tile([HD, ET, DL], F32R, tag="wk")
                wv = wqp.tile([HD, ET, DL], F32R, tag="wv")
                # stream weights per-et so the first matmul starts immediately
                for et in range(ET):
                    nc.sync.dma_start(wq[:, et, :], wqT[et])
                    nc.sync.dma_start(wk[:, et, :], wkT[et])
                    nc.sync.dma_start(wv[:, et, :], wvT[et])

                for b in range(B):
                    qsb[b] = [qkp.tile([HD, N], F32R, tag=f"q{h}",
                                       name=f"qs{b}{h}")
                              for h in range(HL)]
                    ksb[b] = [qkp.tile([HD, N], F32R, tag=f"k{h}",
                                       name=f"ks{b}{h}")
                              for h in range(HL)]
                    vsb[b] = vsp.tile([HD, NT, DL], BF16, tag="v",
                                      name=f"vs{b}")
                    for c4 in range(NCH):
                        nsl = slice(c4 * 512, (c4 + 1) * 512)
                        xts = []
                        for et in range(ET):
                            xt = xp.tile([HD, 512], F32R, tag="xt",
                                         name=f"xt{et}")
                            nc.sync.dma_start(
                                xt[:], xT[b, et * HD:(et + 1) * HD, nsl])
                            xts.append(xt)
                        for h in range(HL):
                            psq = pq.tile([HD, 512], F32, tag="pq")
                            psk = pq.tile([HD, 512], F32, tag="pk")
                            psv = [pq.tile([HD, DL], F32, tag=f"pv{i}",
                                           name=f"psv{i}") for i in range(2)]
                            for et in range(ET):
                                st = (et == 0)
                                en = (et == ET - 1)
                                nc.tensor.matmul(
                                    psq[:], wq[:, et, h * HD:(h + 1) * HD],
                                    xts[et][:], start=st, stop=en)
                                nc.tensor.matmul(
                                    psk[:], wk[:, et, h * HD:(h + 1) * HD],
                                    xts[et][:], start=st, stop=en)
                                for i in range(2):
                                    s4 = 2 * h + i
                                    nc.tensor.matmul(
                                        psv[i][:],
                                        xts[et][:, s4 * HD:(s4 + 1) * HD],
                                        wv[:, et, :], start=st, stop=en)
                            # drain psv to bf16 V via scalar engine (idle here)
                            for i in range(2):
                                nc.scalar.copy(
                                    vsb[b][:, c4 * 4 + 2 * h + i, :],
                                    psv[i][:])
                            # copy-first RoPE: scalar engine drains the PSUM
                            # bank, DVE rotates in SBUF, result lands in the
                            # resident q/k tile (tables half-duplicated so
                            # every tensor_tensor has equal partition bases)
                            for src, dst in ((psq, qsb[b][h]),
                                             (psk, ksb[b][h])):
                                stg = sp.tile([HD, 512], F32R, tag="stg")
                                t12 = sp.tile([HD, 512], F32, tag="t12")
                                nc.scalar.copy(stg[:], src[:])
                                nc.vector.tensor_mul(
                                    t12[:HALF, :], stg[HALF:, :],
                                    sinb[HALF:, nsl])
                                nc.vector.tensor_mul(
                                    t12[HALF:, :], stg[:HALF, :],
                                    sinb[:HALF, nsl])
                                nc.vector.tensor_mul(stg[:], stg[:],
                                                     cosb[:, nsl])
                                nc.vector.tensor_sub(dst[:HALF, nsl],
                                                     stg[:HALF, :],
                                                     t12[:HALF, :])
                                nc.vector.tensor_add(dst[HALF:, nsl],
                                                     stg[HALF:, :],
                                                     t12[HALF:, :])

            # ------------- Phase 2: attention per (h, b) + AllToAll --------
            # proj weights prefetch during attention (x/wqkv SBUF now free)
            wpctx = tc.tile_pool(name="projw", bufs=1)
            wpp = wpctx.__enter__()
            wpt = [wpp.tile([HD, ET, 512], BF16, tag=f"wpt{f}",
                            name=f"wpt{f}") for f in range(4)]
            for f4 in range(4):
                nc.sync.dma_start(wpt[f4][:],
                                  wpTb[:, :, f4 * 512:(f4 + 1) * 512])

            with tc.tile_pool(name="atts", bufs=4) as sp2, \
                 tc.tile_pool(name="attps", bufs=2, space="PSUM") as pqs, \
                 tc.tile_pool(name="attpo", bufs=2, space="PSUM") as pqo, \
                 tc.tile_pool(name="attpd", bufs=2, space="PSUM") as pqd:
                # h=1 first: its A2A overlaps the rest of attention, and the
                # projection accumulates h=1 sources first
                for b in range(B):
                    for h in (1, 0):
                        qt = qsb[b][h]
                        kt = ksb[b][h]
                        vt = vsb[b]
                        for c4 in range(NCH):
                            q0 = c4 * 512
                            pso = pqo.tile([HD, 512], F32, tag="pso")
                            acc = sp2.tile([HD, 512], BF16, tag="acc")
                            njt = 4 * c4 + 4
                            for jt in range(njt):
                                # diagonal tiles (jt = 4*c4+r, r>0) have no
                                # valid columns below 128*r: stream only the
                                # live range
                                r = jt - 4 * c4
                                lo = 128 * r if r > 0 else 0
                                pss = pqs.tile([HD, 512], F32, tag="pss")
                                nc.tensor.matmul(
                                    pss[:, lo:], kt[:, jt * HD:(jt + 1) * HD],
                                    qt[:, q0 + lo:q0 + 512],
                                    start=True, stop=True)
                                pt = sp2.tile([HD, 512], BF16, tag="pt")
                                nc.scalar.activation(
                                    pt[:, lo:], pss[:, lo:], AF.Exp,
                                    bias=kpad[:, b, jt:jt + 1], scale=SCALE)
                                if r >= 0:
                                    # zero the upper triangle of the single
                                    # 128-wide diagonal sub-block
                                    nc.vector.tensor_mul(
                                        pt[:, lo:lo + HD], pt[:, lo:lo + HD],
                                        trim[:])
                                if jt == 0:
                                    nc.vector.tensor_copy(acc[:], pt[:])
                                else:
                                    nc.vector.tensor_add(acc[:, lo:],
                                                         acc[:, lo:],
                                                         pt[:, lo:])
                                nc.tensor.matmul(
                                    pso[:, lo:],
                                    vt[:, jt, h * HD:(h + 1) * HD],
                                    pt[:, lo:],
                                    start=(jt == 0), stop=(jt == njt - 1))
                            psd = pqd.tile([1, 512], F32, tag="psd")
                            nc.tensor.matmul(psd[:], ones[:], acc[:],
                                             start=True, stop=True)
                            rec = sp2.tile([1, 512], F32, tag="rec")
                            nc.vector.reciprocal_approx_fast(rec[:], psd[:])
                            rb = sp2.tile([HD, 512], F32, tag="rb")
                            nc.gpsimd.partition_broadcast(rb[:], rec[:])
                            ou = sp2.tile([HD, 512], BF16, tag="ou")
                            nc.vector.tensor_mul(ou[:], pso[:], rb[:])
                            for rr in range(2):
                                nc.sync.dma_start(
                                    ca[h][b][2 * c4 + rr],
                                    ou[:, rr * NS:(rr + 1) * NS])
                        # exchange this (head, batch) while the next pair
                        # computes
                        nc.gpsimd.collective_compute(
                            "AllToAll", mybir.AluOpType.bypass,
                            replica_groups=[list(range(NCORES))],
                            ins=[ca[h][b].opt()], outs=[cb[h][b].opt()])

            # ---------------- Phase 3: output projection ----------------
            with tc.tile_pool(name="proj", bufs=1) as pj, \
                 tc.tile_pool(name="projs", bufs=2) as po, \
                 tc.tile_pool(name="projp", bufs=2, space="PSUM") as pqp:
                # bias row broadcast to all 128 partitions
                bp1 = pj.tile([1, D], F32)
                nc.sync.dma_start(bp1[:], bp[:])
                bpb = pj.tile([HD, D], F32)
                nc.gpsimd.partition_broadcast(bpb[:], bp1[:])
                # asb[h][b][p, s, n] = cb[h][b][s, p, n]
                asb = [[pj.tile([HD, NCORES, NS], BF16, name=f"asb{h}{b}")
                        for b in range(B)] for h in range(HL)]
                for b in range(B):
                    for h in (1, 0):
                        for s in range(NCORES):
                            nc.sync.dma_start(asb[h][b][:, s], cb[h][b][s])
                for b in range(B):
                    for f4 in range(4):
                        fsl = slice(f4 * 512, (f4 + 1) * 512)
                        for m in range(NS // HD):
                            psp = pqp.tile([HD, 512], F32, tag="psp")
                            # h=1 tiles first: they are exchanged earlier, so
                            # accumulation can begin while h=0 is in flight
                            for gi in range(ET):
                                h = 1 - gi // NCORES
                                s = gi % NCORES
                                nc.tensor.matmul(
                                    psp[:],
                                    asb[h][b][:, s, m * HD:(m + 1) * HD],
                                    wpt[f4][:, 2 * s + h, :],
                                    start=(gi == 0), stop=(gi == ET - 1))
                            ot = po.tile([HD, 512], F32, tag="ot")
                            nc.vector.tensor_add(ot[:], psp[:], bpb[:, fsl])
                            nc.sync.dma_start(
                                out[b, m * HD:(m + 1) * HD, fsl], ot[:])
            wpctx.__exit__(None, None, None)
            vsctx.__exit__(None, None, None)
            qkctx.__exit__(None, None, None)

    nc.compile()
    _CACHE["nc"] = nc
    return nc


def _prep_inputs(x, Wqkv, Wproj, bproj, L):
    x = np.asarray(x, np.float32)
    Wqkv = np.asarray(Wqkv, np.float32)
    Wproj = np.asarray(Wproj, np.float32)
    bproj = np.asarray(bproj, np.float32)
    L = np.asarray(L, np.int32)

    xT = np.ascontiguousarray(x.transpose(0, 2, 1))
    wpTb = np.ascontiguousarray(
        Wproj.T.reshape(ET, HD, D).transpose(1, 0, 2)).astype(
            ml_dtypes.bfloat16)
    inv = 1.0 / (ROPE_BASE ** (np.arange(0, HD, 2, dtype=np.float32) / HD))
    ang = np.arange(N, dtype=np.float32)[:, None] * inv[None, :]
    cos1 = np.cos(ang).T.astype(np.float32)          # [64, N]
    sin1 = np.sin(ang).T.astype(np.float32)
    cosT = np.ascontiguousarray(np.vstack([cos1, cos1]))   # [128, N]
    sinT = np.ascontiguousarray(np.vstack([sin1, sin1]))
    Lw = L.reshape(1, B).astype(np.int32)
    bp = bproj.reshape(1, D)
    trimask = np.triu(np.ones((HD, HD), np.float32)).astype(ml_dtypes.bfloat16)

    in_maps = []
    for c in range(NCORES):
        sl = slice(c * DL, (c + 1) * DL)
        # [ET, HD, DL]: per-et transposed slices of each weight third
        w3 = [np.ascontiguousarray(
                  Wqkv[i * D:(i + 1) * D][sl].T.reshape(ET, HD, DL))
              for i in range(3)]
        in_maps.append({
            "xT": xT, "wqT": w3[0], "wkT": w3[1], "wvT": w3[2],
            "wpTb": wpTb, "bp": bp,
            "cosT": cosT, "sinT": sinT, "Lw": Lw,
            "onesv": np.ones((HD, 1), ml_dtypes.bfloat16),
            "trimT": trimask,
        })
    return in_maps


def run(x, Wqkv, Wproj, bproj, L, trace=False, tmpdir=None):
    nc = build()
    in_maps = _prep_inputs(x, Wqkv, Wproj, bproj, L)
    kw = {}
    if tmpdir is not None:
        kw["tmpdir"] = tmpdir
    res = bass_utils.run_bass_kernel_spmd(
        nc, in_maps, core_ids=list(range(NCORES)), trace=trace, **kw)
    full = np.empty((B, N, D), np.float32)
    for c in range(NCORES):
        full[:, c * NS:(c + 1) * NS, :] = res.results[c]["out"]
    return full, res


def kernel(x, Wqkv, Wproj, bproj, L, n_heads):
    assert int(n_heads) == NH
    full, _ = run(x, Wqkv, Wproj, bproj, L, trace=False)
    return full


# revision 13
# speedup vs baseline: 1.1582x; 1.0087x over previous
"""Distributed Trainium2 kernel for fused multi-head attention
(QKV proj + RoPE + causal/key-padded SDPA + out-proj + bias).

Sharding: tensor-parallel over heads across 8 cores (2 heads/core, both
batches on every core).  After attention, per-(head,batch) AllToAlls
convert head-shards into sequence-row-shards so the output projection is
computed locally per row slice; the host concatenates the 8 row slices.

v3: software-pipelined windows —
  w1: qkv(b0)
  w2: qkv(b1) interleaved with attn(b0) chunks (+ the two b0 AllToAlls)
  w3: attn(b1) interleaved with proj(b0) psum tiles (+ b1 AllToAlls)
  w4: proj(b1)
so attention's scalar/vector work hides under qkv/proj matmuls, and every
AllToAll except the last overlaps compute.  Q/K stay SBUF-resident in
f32r; V / attn*V / AllToAll payload / out-projection run in bf16.
Weights stream per-et interleaved with the first x tiles so the PE
starts ~2us in.  Softmax denominators accumulate on DVE; causal diagonal
masked by a static bf16 triangular multiply; reciprocal_approx_fast.
"""

import collections

import numpy as np
import ml_dtypes

import concourse.bacc as bacc
import concourse.bass as bass
import concourse.mybir as mybir
import concourse.tile as tile
from concourse import bass_utils

B, N, D, NH = 2, 2048, 2048, 16
HD = 128               # head dim
NCORES = 8
HL = NH // NCORES      # heads per core = 2
DL = HL * HD           # local model cols = 256
NS = N // NCORES       # output row slice per core = 256
HALF = HD // 2
ET = D // HD           # 16 contraction tiles
NT = N // HD           # 16 seq tiles of 128
NCH = N // 512         # 4 free-dim chunks of 512
SCALE = 1.0 / float(np.sqrt(HD))
NEGBIG = -30000.0
ROPE_BASE = 10000.0

F32 = mybir.dt.float32
F32R = mybir.dt.float32r
BF16 = mybir.dt.bfloat16
I32 = mybir.dt.int32

_CACHE = {}


def build():
    if "nc" in _CACHE:
        return _CACHE["nc"]
    nc = bacc.Bacc("TRN2", target_bir_lowering=False, debug=False,
                   num_devices=NCORES)
    xT = nc.dram_tensor("xT", [B, D, N], F32R, kind="ExternalInput")
    wqT = nc.dram_tensor("wqT", [ET, HD, DL], F32R, kind="ExternalInput")
    wkT = nc.dram_tensor("wkT", [ET, HD, DL], F32R, kind="ExternalInput")
    wvT = nc.dram_tensor("wvT", [ET, HD, DL], F32R, kind="ExternalInput")
    wpTb = nc.dram_tensor("wpTb", [HD, ET, D], BF16, kind="ExternalInput")
    bp = nc.dram_tensor("bp", [1, D], BF16, kind="ExternalInput")
    cosT = nc.dram_tensor("cosT", [HD, N], F32, kind="ExternalInput")
    sinT = nc.dram_tensor("sinT", [HD, N], F32, kind="ExternalInput")
    Lw = nc.dram_tensor("Lw", [1, B], I32, kind="ExternalInput")
    trimT = nc.dram_tensor("trimT", [HD, HD], BF16, kind="ExternalInput")
    out = nc.dram_tensor("out", [B, NS, D], F32, kind="ExternalOutput")

    AF = mybir.ActivationFunctionType
    ALU = mybir.AluOpType

    with tile.TileContext(nc) as tc:
        with tc.tile_pool(name="persist", bufs=1) as pp, \
             tc.tile_pool(name="dram", bufs=1, space="DRAM") as dp:
            ca = [[dp.tile([NCORES, HD, NS], BF16, name=f"ca{h}{b}")
                   for b in range(B)] for h in range(HL)]
            cb = [[dp.tile([NCORES, HD, NS], BF16, name=f"cb{h}{b}")
                   for b in range(B)] for h in range(HL)]

            ones = pp.tile([HD, 1], BF16)
            nc.vector.memset(ones[:], 1.0)
            trim = pp.tile([HD, HD], BF16)
            nc.sync.dma_start(trim[:], trimT[:])

            # key-padding additive bias per (partition=j%128, batch, jtile)
            iota = pp.tile([HD, NT], I32)
            nc.gpsimd.iota(iota[:], pattern=[[HD, NT]], base=0,
                           channel_multiplier=1)
            iotaf = pp.tile([HD, NT], F32)
            nc.vector.tensor_copy(iotaf[:], iota[:])
            lsb = pp.tile([1, B], I32)
            nc.sync.dma_start(lsb[:], Lw[:])
            lf = pp.tile([1, B], F32)
            nc.vector.tensor_copy(lf[:], lsb[:])
            lb = pp.tile([HD, B], F32)
            nc.gpsimd.partition_broadcast(lb[:], lf[:])
            kpad = pp.tile([HD, B, NT], F32)
            for b in range(B):
                nc.vector.tensor_scalar(kpad[:, b, :], iotaf[:], lb[:, b:b + 1],
                                        NEGBIG, ALU.is_ge, ALU.mult)

            # rope tables duplicated across both partition halves (their DMAs
            # are emitted inside the weight stream below)
            cosb = pp.tile([HD, N], F32)
            sinb = pp.tile([HD, N], F32)

            # SBUF-resident q/k (f32r) and v (bf16), per batch
            qkctx = tc.tile_pool(name="qk", bufs=2)
            qkp = qkctx.__enter__()
            vsctx = tc.tile_pool(name="vres", bufs=2)
            vsp = vsctx.__enter__()
            qsb, ksb, vsb = {}, {}, {}

            def alloc_qkv(b):
                qsb[b] = [qkp.tile([HD, N], F32R, tag=f"q{h}",
                                   name=f"qs{b}{h}") for h in range(HL)]
                ksb[b] = [qkp.tile([HD, N], F32R, tag=f"k{h}",
                                   name=f"ks{b}{h}") for h in range(HL)]
                vsb[b] = vsp.tile([HD, NT, DL], BF16, tag="v", name=f"vs{b}")

            # attention pools outlive the qkv/proj pools (stack nesting)
            with tc.tile_pool(name="atts", bufs=3) as sp2, \
                 tc.tile_pool(name="attps", bufs=2, space="PSUM") as pqs, \
                 tc.tile_pool(name="attpo", bufs=1, space="PSUM") as pqo, \
                 tc.tile_pool(name="attpd", bufs=1, space="PSUM") as pqd:

                def attn_chunk(b, h, c4):
                    qt, kt, vt = qsb[b][h], ksb[b][h], vsb[b]
                    q0 = c4 * 512
                    pso = pqo.tile([HD, 512], F32, tag="pso", name="pso")
                    acc = sp2.tile([HD, 512], BF16, tag="acc", name="acc")
                    njt = 4 * c4 + 4
                    for jt in range(njt):
                        r = jt - 4 * c4
                        lo = 128 * r if r > 0 else 0
                        pss = pqs.tile([HD, 512], F32, tag="pss", name="pss")
                        nc.tensor.matmul(
                            pss[:, lo:], kt[:, jt * HD:(jt + 1) * HD],
                            qt[:, q0 + lo:q0 + 512], start=True, stop=True)
                        pt = sp2.tile([HD, 512], BF16, tag="pt", name="pt")
                        nc.scalar.activation(
                            pt[:, lo:], pss[:, lo:], AF.Exp,
                            bias=kpad[:, b, jt:jt + 1], scale=SCALE)
                        if r >= 0:
                            nc.vector.tensor_mul(
                                pt[:, lo:lo + HD], pt[:, lo:lo + HD], trim[:])
                        if jt == 0:
                            nc.vector.tensor_copy(acc[:], pt[:])
                        else:
                            nc.vector.tensor_add(acc[:, lo:], acc[:, lo:],
                                                 pt[:, lo:])
                        nc.tensor.matmul(
                            pso[:, lo:], vt[:, jt, h * HD:(h + 1) * HD],
                            pt[:, lo:], start=(jt == 0), stop=(jt == njt - 1))
                    psd = pqd.tile([1, 512], F32, tag="psd", name="psd")
                    nc.tensor.matmul(psd[:], ones[:], acc[:],
                                     start=True, stop=True)
                    rec = sp2.tile([1, 512], F32, tag="rec", name="rec")
                    nc.vector.reciprocal_approx_fast(rec[:], psd[:])
                    rb = sp2.tile([HD, 512], F32, tag="rb", name="rb")
                    nc.gpsimd.partition_broadcast(rb[:], rec[:])
                    ou = sp2.tile([HD, 512], BF16, tag="ou", name="ou")
                    nc.vector.tensor_mul(ou[:], pso[:], rb[:])
                    # gpsimd-queue DMA keeps the sync queue free for x tiles
                    for rr in range(2):
                        nc.gpsimd.dma_start(
                            ca[h][b][2 * c4 + rr], ou[:, rr * NS:(rr + 1) * NS])

                def a2a(b, h):
                    nc.gpsimd.collective_compute(
                        "AllToAll", mybir.AluOpType.bypass,
                        replica_groups=[list(range(NCORES))],
                        ins=[ca[h][b].opt()], outs=[cb[h][b].opt()])

                def drain_attn(queue, n):
                    for _ in range(n):
                        if not queue:
                            return
                        u = queue.popleft()
                        if u[0] == "A":
                            attn_chunk(u[1], u[2], u[3])
                        else:
                            a2a(u[1], u[2])

                # ------------- windows 1+2: QKV (+ attn(b0) interleave) ----
                with tc.tile_pool(name="wqkv", bufs=1) as wqp, \
                     tc.tile_pool(name="ph1x", bufs=16) as xp, \
                     tc.tile_pool(name="ph1s", bufs=2) as sp, \
                     tc.tile_pool(name="ph1p", bufs=1, space="PSUM") as pq:
                    wq = wqp.tile([HD, ET, DL], F32R, tag="wq")
                    wk = wqp.tile([HD, ET, DL], F32R, tag="wk")
                    wv = wqp.tile([HD, ET, DL], F32R, tag="wv")

                    def load_x(b, c4, et_range=None):
                        nsl = slice(c4 * 512, (c4 + 1) * 512)
                        xts = []
                        for et in (et_range if et_range is not None
                                   else range(ET)):
                            xt = xp.tile([HD, 512], F32R, tag="xt",
                                         name=f"xt{et}")
                            nc.sync.dma_start(
                                xt[:], xT[b, et * HD:(et + 1) * HD, nsl])
                            xts.append(xt)
                        return xts

                    # stream weights per-et interleaved with the first chunk
                    # of x so the first matmul group starts ~2us in
                    xts00 = []
                    for et in range(ET):
                        nc.sync.dma_start(wq[:, et, :], wqT[et])
                        nc.sync.dma_start(wk[:, et, :], wkT[et])
                        nc.sync.dma_start(wv[:, et, :], wvT[et])
                        xts00 += load_x(0, 0, [et])
                        if et == 5:
                            nc.sync.dma_start(cosb[:], cosT[:])
                            nc.sync.dma_start(sinb[:], sinT[:])

                    def qkv_group(b, c4, h, xts):
                        nsl = slice(c4 * 512, (c4 + 1) * 512)
                        psq = pq.tile([HD, 512], F32, tag="pq", name="psq")
                        psk = pq.tile([HD, 512], F32, tag="pk", name="psk")
                        psv = [pq.tile([HD, DL], F32, tag=f"pv{i}",
                                       name=f"psv{i}") for i in range(2)]
                        for et in range(ET):
                            st = (et == 0)
                            en = (et == ET - 1)
                            nc.tensor.matmul(
                                psq[:], wq[:, et, h * HD:(h + 1) * HD],
                                xts[et][:], start=st, stop=en)
                            nc.tensor.matmul(
                                psk[:], wk[:, et, h * HD:(h + 1) * HD],
                                xts[et][:], start=st, stop=en)
                            for i in range(2):
                                s4 = 2 * h + i
                                nc.tensor.matmul(
                                    psv[i][:],
                                    xts[et][:, s4 * HD:(s4 + 1) * HD],
                                    wv[:, et, :], start=st, stop=en)
                        for i in range(2):
                            nc.scalar.copy(
                                vsb[b][:, c4 * 4 + 2 * h + i, :], psv[i][:])
                        # copy-first RoPE: scalar engine drains PSUM, DVE
                        # rotates, result lands in the resident q/k tile
                        for src, dst in ((psq, qsb[b][h]), (psk, ksb[b][h])):
                            stg = sp.tile([HD, 512], F32R, tag="stg",
                                          name="stg")
                            t12 = sp.tile([HD, 512], F32, tag="t12",
                                          name="t12")
                            nc.scalar.copy(stg[:], src[:])
                            nc.vector.tensor_mul(
                                t12[:HALF, :], stg[HALF:, :], sinb[HALF:, nsl])
                            nc.vector.tensor_mul(
                                t12[HALF:, :], stg[:HALF, :], sinb[:HALF, nsl])
                            nc.vector.tensor_mul(stg[:], stg[:], cosb[:, nsl])
                            nc.vector.tensor_sub(dst[:HALF, nsl],
                                                 stg[:HALF, :], t12[:HALF, :])
                            nc.vector.tensor_add(dst[HALF:, nsl],
                                                 stg[HALF:, :], t12[HALF:, :])

                    # window 1: qkv(b0) alone
                    alloc_qkv(0)
                    for c4 in range(NCH):
                        xts = xts00 if c4 == 0 else load_x(0, c4)
                        for h in range(HL):
                            qkv_group(0, c4, h, xts)

                    # window 2: qkv(b1) interleaved with attn(b0); h=1 first
                    # so its A2A flies while h=0 computes
                    alloc_qkv(1)
                    aq = collections.deque(
                        [("A", 0, 1, c) for c in range(NCH)] + [("C", 0, 1)] +
                        [("A", 0, 0, c) for c in range(NCH)] + [("C", 0, 0)])
                    total = len(aq)
                    done = 0
                    for c4 in range(NCH):
                        xts = load_x(1, c4)
                        for h in range(HL):
                            qkv_group(1, c4, h, xts)
                            done += 1
                            popped = total - len(aq)
                            drain_attn(aq, done * total // 8 - popped)
                    drain_attn(aq, len(aq))

                # ---------- windows 3+4: attn(b1) + proj ----------
                with tc.tile_pool(name="projw", bufs=2) as wpp, \
                     tc.tile_pool(name="proj", bufs=1) as pj, \
                     tc.tile_pool(name="projs", bufs=2) as po, \
                     tc.tile_pool(name="projp", bufs=2, space="PSUM") as pqp:
                    bp1 = pj.tile([1, D], BF16)
                    nc.sync.dma_start(bp1[:], bp[:])
                    bpb = pj.tile([HD, D], BF16)
                    nc.gpsimd.partition_broadcast(bpb[:], bp1[:])
                    asb = [[pj.tile([HD, NCORES, NS], BF16, name=f"asb{h}{b}")
                            for b in range(B)] for h in range(HL)]

                    def gathers(b):
                        for h in (1, 0):
                            for s in range(NCORES):
                                nc.sync.dma_start(asb[h][b][:, s], cb[h][b][s])

                    wpt_cache = {}

                    def wpt_get(b, f4):
                        if (b, f4) not in wpt_cache:
                            w = wpp.tile([HD, ET, 512], BF16, tag="wpt",
                                         name=f"wpt{b}{f4}")
                            nc.sync.dma_start(
                                w[:], wpTb[:, :, f4 * 512:(f4 + 1) * 512])
                            wpt_cache[(b, f4)] = w
                        return wpt_cache[(b, f4)]

                    def proj_psp(b, f4, m):
                        fsl = slice(f4 * 512, (f4 + 1) * 512)
                        wptf = wpt_get(b, f4)
                        psp = pqp.tile([HD, 512], F32, tag="psp", name="psp")
                        # h=1 sources first: their A2A lands earlier
                        for gi in range(ET):
                            h = 1 - gi // NCORES
                            s = gi % NCORES
                            nc.tensor.matmul(
                                psp[:], asb[h][b][:, s, m * HD:(m + 1) * HD],
                                wptf[:, 2 * s + h, :],
                                start=(gi == 0), stop=(gi == ET - 1))
                        ot = po.tile([HD, 512], F32, tag="ot", name="ot")
                        nc.vector.tensor_add(ot[:], psp[:], bpb[:, fsl])
                        nc.sync.dma_start(out[b, m * HD:(m + 1) * HD, fsl],
                                          ot[:])

                    # window 3: attn(b1) interleaved with proj(b0)
                    gathers(0)
                    for c4 in range(NCH):
                        attn_chunk(1, 1, c4)
                    a2a(1, 1)
                    attn_chunk(1, 0, 0)
                    attn_chunk(1, 0, 1)
                    proj_psp(0, 0, 0)
                    attn_chunk(1, 0, 2)
                    proj_psp(0, 0, 1)
                    attn_chunk(1, 0, 3)
                    a2a(1, 0)
                    for f4 in range(1, NCH):
                        for m in range(NS // HD):
                            proj_psp(0, f4, m)
                    # window 4: proj(b1)
                    gathers(1)
                    for f4 in range(NCH):
                        for m in range(NS // HD):
                            proj_psp(1, f4, m)

            vsctx.__exit__(None, None, None)
            qkctx.__exit__(None, None, None)

    nc.compile()
    _CACHE["nc"] = nc
    return nc


def _prep_inputs(x, Wqkv, Wproj, bproj, L):
    x = np.asarray(x, np.float32)
    Wqkv = np.asarray(Wqkv, np.float32)
    Wproj = np.asarray(Wproj, np.float32)
    bproj = np.asarray(bproj, np.float32)
    L = np.asarray(L, np.int32)

    xT = np.ascontiguousarray(x.transpose(0, 2, 1))
    wpTb = np.ascontiguousarray(
        Wproj.T.reshape(ET, HD, D).transpose(1, 0, 2)).astype(
            ml_dtypes.bfloat16)
    inv = 1.0 / (ROPE_BASE ** (np.arange(0, HD, 2, dtype=np.float32) / HD))
    ang = np.arange(N, dtype=np.float32)[:, None] * inv[None, :]
    cos1 = np.cos(ang).T.astype(np.float32)          # [64, N]
    sin1 = np.sin(ang).T.astype(np.float32)
    cosT = np.ascontiguousarray(np.vstack([cos1, cos1]))   # [128, N]
    sinT = np.ascontiguousarray(np.vstack([sin1, sin1]))
    Lw = L.reshape(1, B).astype(np.int32)
    bp = bproj.reshape(1, D).astype(ml_dtypes.bfloat16)
    trimask = np.triu(np.ones((HD, HD), np.float32)).astype(ml_dtypes.bfloat16)

    in_maps = []
    for c in range(NCORES):
        sl = slice(c * DL, (c + 1) * DL)
        w3 = [np.ascontiguousarray(
                  Wqkv[i * D:(i + 1) * D][sl].T.reshape(ET, HD, DL))
              for i in range(3)]
        in_maps.append({
            "xT": xT, "wqT": w3[0], "wkT": w3[1], "wvT": w3[2],
            "wpTb": wpTb, "bp": bp,
            "cosT": cosT, "sinT": sinT, "Lw": Lw,
            "trimT": trimask,
        })
    return in_maps


def run(x, Wqkv, Wproj, bproj, L, trace=False, tmpdir=None):
    nc = build()
    in_maps = _prep_inputs(x, Wqkv, Wproj, bproj, L)
    kw = {}
    if tmpdir is not None:
        kw["tmpdir"] = tmpdir
    res = bass_utils.run_bass_kernel_spmd(
        nc, in_maps, core_ids=list(range(NCORES)), trace=trace, **kw)
    full = np.empty((B, N, D), np.float32)
    for c in range(NCORES):
        full[:, c * NS:(c + 1) * NS, :] = res.results[c]["out"]
    return full, res


def kernel(x, Wqkv, Wproj, bproj, L, n_heads):
    assert int(n_heads) == NH
    full, _ = run(x, Wqkv, Wproj, bproj, L, trace=False)
    return full


# revision 14
# speedup vs baseline: 1.1989x; 1.0352x over previous
"""Distributed Trainium2 kernel for fused multi-head attention
(QKV proj + RoPE + causal/key-padded SDPA + out-proj + bias).

Sharding: tensor-parallel over heads across 8 cores (2 heads/core, both
batches on every core).  After attention, per-(head,batch) AllToAlls
convert head-shards into sequence-row-shards so the output projection is
computed locally per row slice; the host concatenates the 8 row slices.

v3: software-pipelined windows —
  w1: qkv(b0)
  w2: qkv(b1) interleaved with attn(b0) chunks (+ the two b0 AllToAlls)
  w3: attn(b1) interleaved with proj(b0) psum tiles (+ b1 AllToAlls)
  w4: proj(b1)
so attention's scalar/vector work hides under qkv/proj matmuls, and every
AllToAll except the last overlaps compute.  Q/K stay SBUF-resident in
f32r; V / attn*V / AllToAll payload / out-projection run in bf16.
Weights stream per-et interleaved with the first x tiles so the PE
starts ~2us in.  Softmax denominators accumulate on DVE; causal diagonal
masked by a static bf16 triangular multiply; reciprocal_approx_fast.
"""

import collections

import numpy as np
import ml_dtypes

import concourse.bacc as bacc
import concourse.bass as bass
import concourse.mybir as mybir
import concourse.tile as tile
from concourse import bass_utils

B, N, D, NH = 2, 2048, 2048, 16
HD = 128               # head dim
NCORES = 8
HL = NH // NCORES      # heads per core = 2
DL = HL * HD           # local model cols = 256
NS = N // NCORES       # output row slice per core = 256
HALF = HD // 2
ET = D // HD           # 16 contraction tiles
NT = N // HD           # 16 seq tiles of 128
NCH = N // 512         # 4 free-dim chunks of 512
SCALE = 1.0 / float(np.sqrt(HD))
NEGBIG = -30000.0
ROPE_BASE = 10000.0

F32 = mybir.dt.float32
F32R = mybir.dt.float32r
BF16 = mybir.dt.bfloat16
I32 = mybir.dt.int32

_CACHE = {}


def build():
    if "nc" in _CACHE:
        return _CACHE["nc"]
    nc = bacc.Bacc("TRN2", target_bir_lowering=False, debug=False,
                   num_devices=NCORES)
    xT = nc.dram_tensor("xT", [B, D, N], F32R, kind="ExternalInput")
    wqT = nc.dram_tensor("wqT", [ET, HD, DL], F32R, kind="ExternalInput")
    wkT = nc.dram_tensor("wkT", [ET, HD, DL], F32R, kind="ExternalInput")
    wvT = nc.dram_tensor("wvT", [ET, HD, DL], F32R, kind="ExternalInput")
    wpTb = nc.dram_tensor("wpTb", [HD, ET, D], BF16, kind="ExternalInput")
    bp = nc.dram_tensor("bp", [1, D], BF16, kind="ExternalInput")
    cosT = nc.dram_tensor("cosT", [HD, N], F32, kind="ExternalInput")
    sinT = nc.dram_tensor("sinT", [HD, N], F32, kind="ExternalInput")
    Lw = nc.dram_tensor("Lw", [1, B], I32, kind="ExternalInput")
    trimT = nc.dram_tensor("trimT", [HD, HD], BF16, kind="ExternalInput")
    out = nc.dram_tensor("out", [B, NS, D], F32, kind="ExternalOutput")

    AF = mybir.ActivationFunctionType
    ALU = mybir.AluOpType

    with tile.TileContext(nc) as tc:
        with tc.tile_pool(name="persist", bufs=1) as pp, \
             tc.tile_pool(name="dram", bufs=1, space="DRAM") as dp:
            ca = [[dp.tile([NCORES, HD, NS], BF16, name=f"ca{h}{b}")
                   for b in range(B)] for h in range(HL)]
            cb = [[dp.tile([NCORES, HD, NS], BF16, name=f"cb{h}{b}")
                   for b in range(B)] for h in range(HL)]

            ones = pp.tile([HD, 1], BF16)
            nc.vector.memset(ones[:], 1.0)
            trim = pp.tile([HD, HD], BF16)
            nc.sync.dma_start(trim[:], trimT[:])

            # key-padding additive bias per (partition=j%128, batch, jtile)
            iota = pp.tile([HD, NT], I32)
            nc.gpsimd.iota(iota[:], pattern=[[HD, NT]], base=0,
                           channel_multiplier=1)
            iotaf = pp.tile([HD, NT], F32)
            nc.vector.tensor_copy(iotaf[:], iota[:])
            lsb = pp.tile([1, B], I32)
            nc.sync.dma_start(lsb[:], Lw[:])
            lf = pp.tile([1, B], F32)
            nc.vector.tensor_copy(lf[:], lsb[:])
            lb = pp.tile([HD, B], F32)
            nc.gpsimd.partition_broadcast(lb[:], lf[:])
            kpad = pp.tile([HD, B, NT], F32)
            for b in range(B):
                nc.vector.tensor_scalar(kpad[:, b, :], iotaf[:], lb[:, b:b + 1],
                                        NEGBIG, ALU.is_ge, ALU.mult)

            # rope tables duplicated across both partition halves (their DMAs
            # are emitted inside the weight stream below)
            cosb = pp.tile([HD, N], F32)
            sinb = pp.tile([HD, N], F32)

            # SBUF-resident q/k (f32r) and v (bf16), per batch
            qkctx = tc.tile_pool(name="qk", bufs=2)
            qkp = qkctx.__enter__()
            vsctx = tc.tile_pool(name="vres", bufs=2)
            vsp = vsctx.__enter__()
            qsb, ksb, vsb = {}, {}, {}

            def alloc_qkv(b):
                qsb[b] = [qkp.tile([HD, N], BF16, tag=f"q{h}",
                                   name=f"qs{b}{h}") for h in range(HL)]
                ksb[b] = [qkp.tile([HD, N], BF16, tag=f"k{h}",
                                   name=f"ks{b}{h}") for h in range(HL)]
                vsb[b] = vsp.tile([HD, NT, DL], BF16, tag="v", name=f"vs{b}")

            # attention pools outlive the qkv/proj pools (stack nesting)
            with tc.tile_pool(name="atts", bufs=4) as sp2, \
                 tc.tile_pool(name="attps", bufs=2, space="PSUM") as pqs, \
                 tc.tile_pool(name="attpo", bufs=1, space="PSUM") as pqo, \
                 tc.tile_pool(name="attpd", bufs=1, space="PSUM") as pqd:

                def attn_chunk(b, h, c4):
                    qt, kt, vt = qsb[b][h], ksb[b][h], vsb[b]
                    q0 = c4 * 512
                    pso = pqo.tile([HD, 512], F32, tag="pso", name="pso")
                    acc = sp2.tile([HD, 512], BF16, tag="acc", name="acc")
                    njt = 4 * c4 + 4
                    for jt in range(njt):
                        r = jt - 4 * c4
                        lo = 128 * r if r > 0 else 0
                        pss = pqs.tile([HD, 512], F32, tag="pss", name="pss")
                        nc.tensor.matmul(
                            pss[:, lo:], kt[:, jt * HD:(jt + 1) * HD],
                            qt[:, q0 + lo:q0 + 512], start=True, stop=True)
                        pt = sp2.tile([HD, 512], BF16, tag="pt", name="pt")
                        nc.scalar.activation(
                            pt[:, lo:], pss[:, lo:], AF.Exp,
                            bias=kpad[:, b, jt:jt + 1], scale=SCALE)
                        if r >= 0:
                            nc.vector.tensor_mul(
                                pt[:, lo:lo + HD], pt[:, lo:lo + HD], trim[:])
                        if jt == 0:
                            nc.vector.tensor_copy(acc[:], pt[:])
                        else:
                            nc.vector.tensor_add(acc[:, lo:], acc[:, lo:],
                                                 pt[:, lo:])
                        nc.tensor.matmul(
                            pso[:, lo:], vt[:, jt, h * HD:(h + 1) * HD],
                            pt[:, lo:], start=(jt == 0), stop=(jt == njt - 1))
                    psd = pqd.tile([1, 512], F32, tag="psd", name="psd")
                    nc.tensor.matmul(psd[:], ones[:], acc[:],
                                     start=True, stop=True)
                    rec = sp2.tile([1, 512], F32, tag="rec", name="rec")
                    nc.vector.reciprocal_approx_fast(rec[:], psd[:])
                    rb = sp2.tile([HD, 512], F32, tag="rb", name="rb")
                    nc.gpsimd.partition_broadcast(rb[:], rec[:])
                    ou = sp2.tile([HD, 512], BF16, tag="ou", name="ou")
                    nc.vector.tensor_mul(ou[:], pso[:], rb[:])
                    # gpsimd-queue DMA keeps the sync queue free for x tiles
                    for rr in range(2):
                        nc.gpsimd.dma_start(
                            ca[h][b][2 * c4 + rr], ou[:, rr * NS:(rr + 1) * NS])

                def a2a(b, h):
                    nc.gpsimd.collective_compute(
                        "AllToAll", mybir.AluOpType.bypass,
                        replica_groups=[list(range(NCORES))],
                        ins=[ca[h][b].opt()], outs=[cb[h][b].opt()])

                def drain_attn(queue, n):
                    for _ in range(n):
                        if not queue:
                            return
                        u = queue.popleft()
                        if u[0] == "A":
                            attn_chunk(u[1], u[2], u[3])
                        else:
                            a2a(u[1], u[2])

                # ------------- windows 1+2: QKV (+ attn(b0) interleave) ----
                with tc.tile_pool(name="wqkv", bufs=1) as wqp, \
                     tc.tile_pool(name="ph1x", bufs=18) as xp, \
                     tc.tile_pool(name="ph1s", bufs=3) as sp, \
                     tc.tile_pool(name="ph1p", bufs=1, space="PSUM") as pq:
                    wq = wqp.tile([HD, ET, DL], F32R, tag="wq")
                    wk = wqp.tile([HD, ET, DL], F32R, tag="wk")
                    wv = wqp.tile([HD, ET, DL], F32R, tag="wv")

                    def load_x(b, c4, et_range=None):
                        nsl = slice(c4 * 512, (c4 + 1) * 512)
                        xts = []
                        for et in (et_range if et_range is not None
                                   else range(ET)):
                            xt = xp.tile([HD, 512], F32R, tag="xt",
                                         name=f"xt{et}")
                            nc.sync.dma_start(
                                xt[:], xT[b, et * HD:(et + 1) * HD, nsl])
                            xts.append(xt)
                        return xts

                    # stream weights per-et interleaved with the first chunk
                    # of x so the first matmul group starts ~2us in
                    xts00 = []
                    for et in range(ET):
                        nc.sync.dma_start(wq[:, et, :], wqT[et])
                        nc.sync.dma_start(wk[:, et, :], wkT[et])
                        nc.sync.dma_start(wv[:, et, :], wvT[et])
                        xts00 += load_x(0, 0, [et])
                        if et == 5:
                            nc.sync.dma_start(cosb[:], cosT[:])
                            nc.sync.dma_start(sinb[:], sinT[:])

                    def qkv_group(b, c4, h, xts):
                        nsl = slice(c4 * 512, (c4 + 1) * 512)
                        psq = pq.tile([HD, 512], F32, tag="pq", name="psq")
                        psk = pq.tile([HD, 512], F32, tag="pk", name="psk")
                        psv = [pq.tile([HD, DL], F32, tag=f"pv{i}",
                                       name=f"psv{i}") for i in range(2)]
                        for et in range(ET):
                            st = (et == 0)
                            en = (et == ET - 1)
                            nc.tensor.matmul(
                                psq[:], wq[:, et, h * HD:(h + 1) * HD],
                                xts[et][:], start=st, stop=en)
                            nc.tensor.matmul(
                                psk[:], wk[:, et, h * HD:(h + 1) * HD],
                                xts[et][:], start=st, stop=en)
                            for i in range(2):
                                s4 = 2 * h + i
                                nc.tensor.matmul(
                                    psv[i][:],
                                    xts[et][:, s4 * HD:(s4 + 1) * HD],
                                    wv[:, et, :], start=st, stop=en)
                        for i in range(2):
                            nc.scalar.copy(
                                vsb[b][:, c4 * 4 + 2 * h + i, :], psv[i][:])
                        # copy-first RoPE: scalar engine drains PSUM, DVE
                        # rotates, result lands in the resident q/k tile
                        for src, dst in ((psq, qsb[b][h]), (psk, ksb[b][h])):
                            stg = sp.tile([HD, 512], F32R, tag="stg",
                                          name="stg")
                            t12 = sp.tile([HD, 512], F32, tag="t12",
                                          name="t12")
                            nc.scalar.copy(stg[:], src[:])
                            nc.vector.tensor_mul(
                                t12[:HALF, :], stg[HALF:, :], sinb[HALF:, nsl])
                            nc.vector.tensor_mul(
                                t12[HALF:, :], stg[:HALF, :], sinb[:HALF, nsl])
                            nc.vector.tensor_mul(stg[:], stg[:], cosb[:, nsl])
                            nc.vector.tensor_sub(dst[:HALF, nsl],
                                                 stg[:HALF, :], t12[:HALF, :])
                            nc.vector.tensor_add(dst[HALF:, nsl],
                                                 stg[HALF:, :], t12[HALF:, :])

                    # window 1: qkv(b0) alone
                    alloc_qkv(0)
                    for c4 in range(NCH):
                        xts = xts00 if c4 == 0 else load_x(0, c4)
                        for h in range(HL):
                            qkv_group(0, c4, h, xts)

                    # window 2: qkv(b1) interleaved with attn(b0); h=1 first
                    # so its A2A flies while h=0 computes
                    alloc_qkv(1)
                    aq = collections.deque(
                        [("A", 0, 1, c) for c in range(NCH)] + [("C", 0, 1)] +
                        [("A", 0, 0, c) for c in range(NCH)] + [("C", 0, 0)])
                    total = len(aq)
                    done = 0
                    for c4 in range(NCH):
                        xts = load_x(1, c4)
                        for h in range(HL):
                            qkv_group(1, c4, h, xts)
                            done += 1
                            popped = total - len(aq)
                            drain_attn(aq, min(len(aq), done * total // 6 - popped))
                    drain_attn(aq, len(aq))

                # ---------- windows 3+4: attn(b1) + proj ----------
                with tc.tile_pool(name="projw", bufs=2) as wpp, \
                     tc.tile_pool(name="proj", bufs=1) as pj, \
                     tc.tile_pool(name="projs", bufs=2) as po, \
                     tc.tile_pool(name="projp", bufs=2, space="PSUM") as pqp:
                    bp1 = pj.tile([1, D], BF16)
                    nc.sync.dma_start(bp1[:], bp[:])
                    bpb = pj.tile([HD, D], BF16)
                    nc.gpsimd.partition_broadcast(bpb[:], bp1[:])
                    asb = [[pj.tile([HD, NCORES, NS], BF16, name=f"asb{h}{b}")
                            for b in range(B)] for h in range(HL)]

                    def gathers(b):
                        for h in (1, 0):
                            for s in range(NCORES):
                                nc.sync.dma_start(asb[h][b][:, s], cb[h][b][s])

                    wpt_cache = {}

                    def wpt_get(b, f4):
                        if (b, f4) not in wpt_cache:
                            w = wpp.tile([HD, ET, 512], BF16, tag="wpt",
                                         name=f"wpt{b}{f4}")
                            nc.sync.dma_start(
                                w[:], wpTb[:, :, f4 * 512:(f4 + 1) * 512])
                            wpt_cache[(b, f4)] = w
                        return wpt_cache[(b, f4)]

                    def proj_psp(b, f4, m, hfirst):
                        fsl = slice(f4 * 512, (f4 + 1) * 512)
                        wptf = wpt_get(b, f4)
                        psp = pqp.tile([HD, 512], F32, tag="psp", name="psp")
                        # earliest-exchanged head's sources accumulate first
                        for gi in range(ET):
                            h = hfirst if gi < NCORES else 1 - hfirst
                            s = gi % NCORES
                            nc.tensor.matmul(
                                psp[:], asb[h][b][:, s, m * HD:(m + 1) * HD],
                                wptf[:, 2 * s + h, :],
                                start=(gi == 0), stop=(gi == ET - 1))
                        ot = po.tile([HD, 512], F32, tag="ot", name="ot")
                        nc.vector.tensor_add(ot[:], psp[:], bpb[:, fsl])
                        nc.sync.dma_start(out[b, m * HD:(m + 1) * HD, fsl],
                                          ot[:])

                    # window 3: attn(b1) interleaved with proj(b0).
                    # b1 attention runs h=0 first so its A2A flies earliest;
                    # proj(b1) then accumulates h=0 sources first while the
                    # late (b1,h1) A2A is still in flight.
                    gathers(0)
                    for c4 in range(NCH):
                        attn_chunk(1, 0, c4)
                    a2a(1, 0)
                    attn_chunk(1, 1, 0)
                    attn_chunk(1, 1, 1)
                    proj_psp(0, 0, 0, 1)
                    attn_chunk(1, 1, 2)
                    proj_psp(0, 0, 1, 1)
                    attn_chunk(1, 1, 3)
                    a2a(1, 1)
                    gathers(1)
                    for f4 in range(1, NCH):
                        for m in range(NS // HD):
                            proj_psp(0, f4, m, 1)
                    # window 4: proj(b1)
                    for f4 in range(NCH):
                        for m in range(NS // HD):
                            proj_psp(1, f4, m, 0)

            vsctx.__exit__(None, None, None)
            qkctx.__exit__(None, None, None)

    nc.compile()
    _CACHE["nc"] = nc
    return nc


def _prep_inputs(x, Wqkv, Wproj, bproj, L):
    x = np.asarray(x, np.float32)
    Wqkv = np.asarray(Wqkv, np.float32)
    Wproj = np.asarray(Wproj, np.float32)
    bproj = np.asarray(bproj, np.float32)
    L = np.asarray(L, np.int32)

    xT = np.ascontiguousarray(x.transpose(0, 2, 1))
    wpTb = np.ascontiguousarray(
        Wproj.T.reshape(ET, HD, D).transpose(1, 0, 2)).astype(
            ml_dtypes.bfloat16)
    inv = 1.0 / (ROPE_BASE ** (np.arange(0, HD, 2, dtype=np.float32) / HD))
    ang = np.arange(N, dtype=np.float32)[:, None] * inv[None, :]
    cos1 = np.cos(ang).T.astype(np.float32)          # [64, N]
    sin1 = np.sin(ang).T.astype(np.float32)
    cosT = np.ascontiguousarray(np.vstack([cos1, cos1]))   # [128, N]
    sinT = np.ascontiguousarray(np.vstack([sin1, sin1]))
    Lw = L.reshape(1, B).astype(np.int32)
    bp = bproj.reshape(1, D).astype(ml_dtypes.bfloat16)
    trimask = np.triu(np.ones((HD, HD), np.float32)).astype(ml_dtypes.bfloat16)

    in_maps = []
    for c in range(NCORES):
        sl = slice(c * DL, (c + 1) * DL)
        w3 = [np.ascontiguousarray(
                  Wqkv[i * D:(i + 1) * D][sl].T.reshape(ET, HD, DL))
              for i in range(3)]
        in_maps.append({
            "xT": xT, "wqT": w3[0], "wkT": w3[1], "wvT": w3[2],
            "wpTb": wpTb, "bp": bp,
            "cosT": cosT, "sinT": sinT, "Lw": Lw,
            "trimT": trimask,
        })
    return in_maps


def run(x, Wqkv, Wproj, bproj, L, trace=False, tmpdir=None):
    nc = build()
    in_maps = _prep_inputs(x, Wqkv, Wproj, bproj, L)
    kw = {}
    if tmpdir is not None:
        kw["tmpdir"] = tmpdir
    res = bass_utils.run_bass_kernel_spmd(
        nc, in_maps, core_ids=list(range(NCORES)), trace=trace, **kw)
    full = np.empty((B, N, D), np.float32)
    for c in range(NCORES):
        full[:, c * NS:(c + 1) * NS, :] = res.results[c]["out"]
    return full, res


def kernel(x, Wqkv, Wproj, bproj, L, n_heads):
    assert int(n_heads) == NH
    full, _ = run(x, Wqkv, Wproj, bproj, L, trace=False)
    return full
